# revision 8
# baseline (speedup 1.0000x reference)
"""GCN VGAE encoder (nn_Encoder_25065429139538) on 8 Trainium2 NeuronCores.

Strategy (sharding_hint: shard nodes across cores, partition edges by dst,
replicate weights):
  - Nodes padded to 100352 = 8 x 12544; core d owns dst rows [d*SH, (d+1)*SH).
  - Aggregation commutes with the dense projections: each layer gathers
    projected rows h[src] (dma_gather, 8192 idxs/instruction), scales them by
    the folded GCN norm coefficient (DVE), and scatter-adds per dst block with
    a one-hot matmul on the TensorEngine accumulating in PSUM.
  - Layer tables are stored p-major ([128, 98*64]: node t*128+p at partition p,
    cols t*64..): projections write p-major directly (bias folded into the
    matmul via a ones row), the self-loop stream is read straight from SBUF,
    and the AllGather is split into 4 partition-quarters so the next pass's
    gathers (grouped by src quarter, s-outer sweeps) overlap the collective.
  - mu/logstd share one aggregation pass (both use the unweighted norm on h2).
"""

import math

import numpy as np

import concourse.bass as bass
import concourse.bacc as bacc
import concourse.mybir as mybir
import concourse.tile as tile
from concourse.bass_utils import run_bass_kernel_spmd
from concourse.library_config import mlp

# ---- problem constants (hardcoded per contract) ----
N = 100000
FIN, HID, OUT = 128, 64, 32
NCORES = 8

# ---- layout constants ----
SH = 12544            # rows per core (100352 / 8)
NPAD = SH * NCORES    # 100352
NBLK = SH // 128      # 98 dst blocks per core
NSUB = 4              # src quarters (q = partition // 32); int16 gather idx
QROWS = 32            # partition rows per quarter
SUBR = NPAD // NSUB   # 25088 rows per quarter table
SLOTS = 8192          # gather slots per dma_gather instruction
CPG = SLOTS // 128    # chunks per gather group = 64
HCPG = CPG // 2       # one-hot built in halves (iota tile stays small)


def _wrap_idx(slots_i16):
    """[G*SLOTS] int16 -> [G, 128, SLOTS//16]: wrapped in 16 partitions, x8."""
    g = slots_i16.reshape(-1, SLOTS // 16, 16)
    g = np.swapaxes(g, 1, 2)
    return np.tile(g, (1, 8, 1)).astype(np.int16)


def _prep(edge_index, edge_weight):
    """Host-side edge partitioning. Returns (structure, per-core arrays)."""
    src = np.asarray(edge_index[0], dtype=np.int64)
    dst = np.asarray(edge_index[1], dtype=np.int64)
    ew = np.asarray(edge_weight, dtype=np.float32)

    deg_w = np.zeros(N, np.float32)
    np.add.at(deg_w, dst, ew)
    deg_w += 1.0  # self-loop weight
    deg_1 = (np.bincount(dst, minlength=N) + 1).astype(np.float32)
    dinv_w = 1.0 / np.sqrt(deg_w)
    dinv_1 = 1.0 / np.sqrt(deg_1)

    nw = dinv_w[src] * ew * dinv_w[dst]
    n1 = dinv_1[src] * dinv_1[dst]

    # merge duplicate (src, dst) pairs (self-loops handled via dense stream)
    key = src * NPAD + dst
    ukey, inv = np.unique(key, return_inverse=True)
    unw = np.zeros(len(ukey), np.float32)
    un1 = np.zeros(len(ukey), np.float32)
    np.add.at(unw, inv, nw)
    np.add.at(un1, inv, n1)
    usrc = ukey // NPAD
    udst = ukey % NPAD

    # src side: p-major quarter-table position
    sd = usrc // SH
    st_ = (usrc % SH) // 128
    sp_ = (usrc % SH) % 128
    s_all = sp_ // QROWS
    sidx_all = sd * (QROWS * NBLK) + (sp_ % QROWS) * NBLK + st_  # < 25088

    # dst side
    core = udst // SH
    t_all = (udst % SH) // 128
    dloc_all = (udst % SH) % 128

    # per (core, t, s) edge counts -> shared chunk structure K_ts
    cell = (core * NBLK + t_all) * NSUB + s_all
    cnt = np.bincount(cell, minlength=NCORES * NBLK * NSUB).reshape(NCORES, NBLK, NSUB)
    K_ts = np.maximum(1, np.ceil(cnt.max(axis=0) / 128).astype(np.int64))  # [NBLK, NSUB]
    C_s = K_ts.sum(axis=0)                      # chunks per s-stream
    G_s = [int(math.ceil(int(c) / CPG)) for c in C_s]
    base_pos = np.zeros((NBLK, NSUB), np.int64)  # chunk stream position of (t,s)
    for s in range(NSUB):
        base_pos[:, s] = np.cumsum(np.concatenate([[0], K_ts[:-1, s]]))

    per_core = []
    order = np.lexsort((sidx_all, t_all, s_all, core))
    osidx, os_, ot, odloc, onw, on1, ocore = (
        sidx_all[order], s_all[order], t_all[order],
        dloc_all[order], unw[order], un1[order], core[order],
    )
    cstart = np.searchsorted(ocore, np.arange(NCORES + 1))
    for d in range(NCORES):
        lo, hi = cstart[d], cstart[d + 1]
        dt, ds = ot[lo:hi], os_[lo:hi]
        dsl, ddl = osidx[lo:hi], odloc[lo:hi]
        dnw, dn1 = onw[lo:hi], on1[lo:hi]
        idxw, dlocv, normv = [], [], []
        cell_d = ds * NBLK + dt
        cello = np.searchsorted(cell_d, np.arange(NSUB * NBLK + 1))
        for s in range(NSUB):
            nslot = G_s[s] * SLOTS
            sl = np.zeros(nslot, np.int64)
            dl = np.zeros(nslot, np.float32)
            wv = np.zeros(nslot, np.float32)
            v1 = np.zeros(nslot, np.float32)
            for t in range(NBLK):
                a, b = cello[s * NBLK + t], cello[s * NBLK + t + 1]
                n = b - a
                p0 = base_pos[t, s] * 128
                sl[p0:p0 + n] = dsl[a:b]
                dl[p0:p0 + n] = ddl[a:b]
                wv[p0:p0 + n] = dnw[a:b]
                v1[p0:p0 + n] = dn1[a:b]
            sl[int(C_s[s]) * 128:] = -1  # trailing: ucode skips descriptors
            G = G_s[s]
            idxw.append(_wrap_idx(sl.astype(np.int16)))
            dlocv.append(
                dl.reshape(G, CPG, 128).transpose(0, 2, 1).astype(np.float16).copy()
            )
            nv = np.empty((G, 128, 2 * CPG), np.float32)
            nv[:, :, :CPG] = wv.reshape(G, CPG, 128).transpose(0, 2, 1)
            nv[:, :, CPG:] = v1.reshape(G, CPG, 128).transpose(0, 2, 1)
            normv.append(nv)
        # self-loop norms, p-major: [p, t] = dinv^2 of row d*SH + t*128 + p
        selfn = np.zeros((128, 2 * NBLK), np.float32)
        v_glob = (d * SH + np.arange(SH, dtype=np.int64)).reshape(NBLK, 128)
        real = v_glob < N
        sw = np.zeros((NBLK, 128), np.float32)
        s1 = np.zeros((NBLK, 128), np.float32)
        sw[real] = (dinv_w * dinv_w)[v_glob[real]]
        s1[real] = (dinv_1 * dinv_1)[v_glob[real]]
        selfn[:, :NBLK] = sw.T
        selfn[:, NBLK:] = s1.T
        per_core.append((idxw, dlocv, normv, selfn))

    used_s = [
        [min(SLOTS, int(C_s[s]) * 128 - g * SLOTS) for g in range(G_s[s])]
        for s in range(NSUB)
    ]
    return K_ts, G_s, base_pos, used_s, per_core


def _build(K_ts, G_s, base_pos, used_s):
    f32 = mybir.dt.float32
    f16 = mybir.dt.float16
    i16 = mybir.dt.int16
    nc = bacc.Bacc(None, target_bir_lowering=False, num_swdge_queues=4, num_devices=NCORES)

    xs_d = nc.dram_tensor("xs", [SH, FIN], f32, kind="ExternalInput")
    idx_d = [nc.dram_tensor(f"idx{s}", [G_s[s], 128, SLOTS // 16], i16, kind="ExternalInput") for s in range(NSUB)]
    dloc_d = [nc.dram_tensor(f"dloc{s}", [G_s[s], 128, CPG], f16, kind="ExternalInput") for s in range(NSUB)]
    norm_d = [nc.dram_tensor(f"norm{s}", [G_s[s], 128, 2 * CPG], f32, kind="ExternalInput") for s in range(NSUB)]
    selfn_d = nc.dram_tensor("selfn", [128, 2 * NBLK], f32, kind="ExternalInput")
    w1_d = nc.dram_tensor("w1h", [FIN, HID], f16, kind="ExternalInput")
    rl_d = nc.dram_tensor("rlr", [HID + 1, HID], f16, kind="ExternalInput")
    w2r_d = nc.dram_tensor("w2r", [HID + 1, HID], f16, kind="ExternalInput")
    w3r_d = nc.dram_tensor("w3r", [HID + 1, 2 * OUT], f16, kind="ExternalInput")
    iota_d = nc.dram_tensor("iota", [128, HCPG * 128], f16, kind="ExternalInput")
    id32_d = nc.dram_tensor("id32", [128, 128], f32, kind="ExternalInput")
    id16_d = nc.dram_tensor("id16", [128, 128], f16, kind="ExternalInput")
    ones_d = nc.dram_tensor("ones", [1, NBLK * 128], f16, kind="ExternalInput")
    out_d = nc.dram_tensor("out", [128, NBLK * 2 * OUT], f16, kind="ExternalOutput")

    ag_in = [[nc.dram_tensor(f"ag{i}_{q}", [QROWS, NBLK * HID], f32) for q in range(NSUB)] for i in range(3)]
    tables = [[nc.dram_tensor(f"tab{i}_{q}", [SUBR, HID], f32, addr_space="Shared") for q in range(NSUB)] for i in range(3)]

    with tile.TileContext(nc) as tc:
        with (
            tc.tile_pool(name="const", bufs=1) as kpool,
            tc.tile_pool(name="idx", bufs=3) as ipool,
            tc.tile_pool(name="meta", bufs=3) as mpool,
            tc.tile_pool(name="g", bufs=2) as gpool,
            tc.tile_pool(name="gh", bufs=2) as hpool,
            tc.tile_pool(name="b", bufs=2) as bpool,
            tc.tile_pool(name="st", bufs=2) as spool,
            tc.tile_pool(name="selfd", bufs=1) as dpool,
            tc.tile_pool(name="o", bufs=1) as opool,
            tc.tile_pool(name="x", bufs=2) as xpool,
            tc.tile_pool(name="pagg", bufs=2, space="PSUM") as pagg,
            tc.tile_pool(name="pmm", bufs=2, space="PSUM") as pmm,
            tc.tile_pool(name="ptr", bufs=2, space="PSUM") as ptr,
        ):
            nc.gpsimd.load_library(mlp)

            iota_t = kpool.tile([128, HCPG * 128], f16)
            nc.sync.dma_start(iota_t[:], iota_d[:])
            id32_t = kpool.tile([128, 128], f32)
            nc.sync.dma_start(id32_t[:], id32_d[:])
            id16_t = kpool.tile([128, 128], f16)
            nc.sync.dma_start(id16_t[:], id16_d[:])
            w1_t = kpool.tile([FIN, HID], f16)
            nc.sync.dma_start(w1_t[:], w1_d[:])
            rl_t = kpool.tile([HID + 1, HID], f16)
            nc.sync.dma_start(rl_t[:], rl_d[:])
            w2r_t = kpool.tile([HID + 1, HID], f16)
            nc.sync.dma_start(w2r_t[:], w2r_d[:])
            w3r_t = kpool.tile([HID + 1, 2 * OUT], f16)
            nc.sync.dma_start(w3r_t[:], w3r_d[:])
            selfn_t = kpool.tile([128, 2 * NBLK], f32)
            nc.sync.dma_start(selfn_t[:], selfn_d[:])
            # aggregation accumulator with a constant ones row (bias input)
            agg_t = kpool.tile([HID + 1, NBLK * 128], f16)
            nc.sync.dma_start(agg_t[HID:HID + 1, :], ones_d[:])

            gq = [0]

            def ensure(cur, i, s, g, use_n1):
                if s in cur and cur[s][0] == g:
                    return cur[s][1]
                it = ipool.tile([128, SLOTS // 16], i16, tag="idx")
                nc.sync.dma_start(it[:], idx_d[s][g])
                dt_ = mpool.tile([128, CPG], f16, tag="dl")
                nc.sync.dma_start(dt_[:], dloc_d[s][g])
                nt = mpool.tile([128, 2 * CPG], f32, tag="nm")
                nc.sync.dma_start(nt[:], norm_d[s][g])
                gt = gpool.tile([128, CPG, HID], f32, tag="gt")
                nc.gpsimd.dma_gather(
                    gt[:], tables[i][s][:], it[:],
                    SLOTS, int(used_s[s][g]), HID, queue_num=gq[0] % 4,
                    single_packet=False,
                )
                gq[0] += 1
                uc = (int(used_s[s][g]) + 127) // 128  # chunks actually gathered
                nsl = nt[:, CPG:CPG + uc] if use_n1 else nt[:, :uc]
                gh = hpool.tile([128, CPG, HID], f16, tag="gh")
                nc.vector.tensor_tensor(
                    out=gh[:, :uc, :], in0=gt[:, :uc, :],
                    in1=nsl.to_broadcast([128, uc, HID]),
                    op=mybir.AluOpType.mult,
                )
                bt = bpool.tile([128, CPG, 128], f16, tag="bt")
                for h in range(2):
                    nc.vector.tensor_tensor(
                        out=bt[:, h * HCPG:(h + 1) * HCPG, :],
                        in0=iota_t[:].rearrange("p (j v) -> p j v", j=HCPG),
                        in1=dt_[:, h * HCPG:(h + 1) * HCPG].to_broadcast([128, HCPG, 128]),
                        op=mybir.AluOpType.is_equal,
                    )
                cur[s] = (g, (gh, bt))
                return gh, bt

            def aggregate(i, st_prev, use_n1):
                """agg_t[:HID, t*128+p] = sum of coeff*table_i[src] into (t,p)."""
                dsh = dpool.tile([128, NBLK, HID], f16, tag="dsh")
                sn = selfn_t[:, NBLK:] if use_n1 else selfn_t[:, :NBLK]
                nc.vector.tensor_tensor(
                    out=dsh[:],
                    in0=st_prev[:].rearrange("p (t f) -> p t f", f=HID),
                    in1=sn.to_broadcast([128, NBLK, HID]),
                    op=mybir.AluOpType.mult,
                )
                cur = {}
                for s in range(NSUB):
                    for t in range(NBLK):
                        ps = pagg.tile([HID, 128], f32, tag="ps")
                        nchunks = int(K_ts[t, s])
                        if s == 0:
                            nc.tensor.matmul(
                                ps[:], lhsT=dsh[:, t, :], rhs=id16_t[:],
                                start=True, stop=False,
                            )
                        for k in range(nchunks):
                            pos = int(base_pos[t, s]) + k
                            g, j = divmod(pos, CPG)
                            gh, bt = ensure(cur, i, s, g, use_n1)
                            nc.tensor.matmul(
                                ps[:], lhsT=gh[:, j, :], rhs=bt[:, j, :],
                                start=(s != 0 and k == 0),
                                stop=(k == nchunks - 1),
                            )
                        dst = agg_t[:HID, t * 128:(t + 1) * 128]
                        if s == 0:
                            nc.scalar.activation(dst, ps[:], mybir.ActivationFunctionType.Copy)
                        else:
                            nc.vector.tensor_tensor(out=dst, in0=dst, in1=ps[:], op=mybir.AluOpType.add)

            def project(rhs_t, func, st_out, outw):
                """st_out[p, t*outw+f] = func((agg | ones).T @ rhs)[p, f]."""
                for t in range(NBLK):
                    pm = pmm.tile([128, outw], f32, tag="pm")
                    nc.tensor.matmul(
                        pm[:], lhsT=agg_t[:, t * 128:(t + 1) * 128], rhs=rhs_t[:],
                        start=True, stop=True,
                    )
                    nc.scalar.activation(st_out[:, t * outw:(t + 1) * outw], pm[:], func)

            def stores_and_ag(i, st_t):
                for q in range(NSUB):
                    nc.sync.dma_start(ag_in[i][q][:], st_t[q * QROWS:(q + 1) * QROWS, :])
                    nc.gpsimd.collective_compute(
                        "AllGather", mybir.AluOpType.bypass,
                        replica_groups=[list(range(NCORES))],
                        ins=[ag_in[i][q][:]], outs=[tables[i][q][:]],
                    )

            # ---- pre-projection: st0 = (x @ W1) p-major ----
            st0 = spool.tile([128, NBLK * HID], f32, tag="st")
            for t in range(NBLK):
                xt = xpool.tile([128, FIN], f32, tag="xt")
                nc.sync.dma_start(xt[:], xs_d[t * 128:(t + 1) * 128, :])
                ptx = ptr.tile([128, 128], f32, tag="ptx")
                nc.tensor.transpose(ptx[:], xt[:], id32_t[:])
                xT = xpool.tile([128, 128], f16, tag="xT")
                nc.vector.tensor_copy(xT[:], ptx[:])
                ph = pmm.tile([128, HID], f32, tag="pm")
                nc.tensor.matmul(ph[:], lhsT=xT[:], rhs=w1_t[:], start=True, stop=True)
                nc.scalar.activation(st0[:, t * HID:(t + 1) * HID], ph[:], mybir.ActivationFunctionType.Copy)
            stores_and_ag(0, st0)

            # ---- layer 1: aggregate projected x, then bias+relu ----
            aggregate(0, st0, use_n1=False)
            st1 = spool.tile([128, NBLK * HID], f32, tag="st")
            project(rl_t, mybir.ActivationFunctionType.Relu, st1, HID)
            stores_and_ag(1, st1)

            # ---- layer 2: aggregate h1, then W2 + bias ----
            aggregate(1, st1, use_n1=False)
            st2 = spool.tile([128, NBLK * HID], f32, tag="st")
            project(w2r_t, None or mybir.ActivationFunctionType.Copy, st2, HID)
            stores_and_ag(2, st2)

            # ---- layer 3: aggregate h2; mu/ls joint projection ----
            aggregate(2, st2, use_n1=True)
            st3 = opool.tile([128, NBLK * 2 * OUT], f16, tag="st3")
            project(w3r_t, mybir.ActivationFunctionType.Copy, st3, 2 * OUT)
            nc.sync.dma_start(out_d[:], st3[:])

    # Tile round-robins Pool-DMA completion sems over 8 DMASW lanes without
    # queue awareness, but each sem is hardware-locked to the first SWDGE
    # queue that increments it. Rewrite each gather's queue to lane % 4 so
    # every lane's sem is only ever incremented from one queue.
    for fn in nc.m.functions:
        for blk in fn.blocks:
            for ins in blk.instructions:
                if isinstance(ins, mybir.InstDMAGatherAnt) and ins.sync_info:
                    for u in ins.sync_info.on_update:
                        name = getattr(u, "ant_name", "") or ""
                        if name.startswith("DMASW"):
                            ins.queue_num = int(name[5:].split("_")[0]) % 4
                            break

    nc.compile()
    return nc


def _run(inputs, trace=False):
    x = np.asarray(inputs["x"], np.float32)
    K_ts, G_s, base_pos, used_s, per_core = _prep(
        np.asarray(inputs["edge_index"]), np.asarray(inputs["edge_weight"])
    )
    nc = _build(K_ts, G_s, base_pos, used_s)

    x_pad = np.zeros((NPAD, FIN), np.float32)
    x_pad[:N] = x

    W1 = np.asarray(inputs["W1"], np.float32)
    W2 = np.asarray(inputs["W2"], np.float32)
    Wmu = np.asarray(inputs["Wmu"], np.float32)
    Wls = np.asarray(inputs["Wls"], np.float32)
    b1 = np.asarray(inputs["b1"], np.float32)
    b2 = np.asarray(inputs["b2"], np.float32)
    bmu = np.asarray(inputs["bmu"], np.float32)
    bls = np.asarray(inputs["bls"], np.float32)

    rl = np.vstack([np.eye(HID, dtype=np.float32), b1[None, :]]).astype(np.float16)
    w2r = np.vstack([W2, b2[None, :]]).astype(np.float16)
    w3r = np.vstack(
        [np.hstack([Wmu, Wls]), np.hstack([bmu, bls])[None, :]]
    ).astype(np.float16)
    iota = np.tile(np.arange(128, dtype=np.float16)[None, :], (128, HCPG))

    shared = {
        "w1h": W1.astype(np.float16),
        "rlr": rl,
        "w2r": w2r,
        "w3r": w3r,
        "iota": iota.reshape(128, HCPG * 128),
        "id32": np.eye(128, dtype=np.float32),
        "id16": np.eye(128, dtype=np.float16),
        "ones": np.ones((1, NBLK * 128), np.float16),
    }
    in_maps = []
    for d in range(NCORES):
        idxw, dlocv, normv, selfn = per_core[d]
        m = dict(shared)
        m["xs"] = x_pad[d * SH:(d + 1) * SH]
        m["selfn"] = selfn
        for s in range(NSUB):
            m[f"idx{s}"] = idxw[s]
            m[f"dloc{s}"] = dlocv[s]
            m[f"norm{s}"] = normv[s]
        in_maps.append(m)

    res = run_bass_kernel_spmd(nc, in_maps, core_ids=list(range(NCORES)), trace=trace)
    # out: [128, NBLK*2*OUT] p-major -> rows t*128+p
    full = np.concatenate(
        [
            res.results[d]["out"]
            .reshape(128, NBLK, 2 * OUT)
            .transpose(1, 0, 2)
            .reshape(SH, 2 * OUT)
            for d in range(NCORES)
        ],
        axis=0,
    ).astype(np.float32)
    mu = full[:N, :OUT].copy()
    logstd = full[:N, OUT:].copy()
    return (mu, logstd), res


def kernel(**inputs):
    (mu, logstd), _ = _run(inputs, trace=False)
    return mu, logstd


# revision 12
# speedup vs baseline: 1.1687x; 1.1687x over previous
"""GCN VGAE encoder (nn_Encoder_25065429139538) on 8 Trainium2 NeuronCores.

Strategy (sharding_hint: shard nodes across cores, partition edges by dst,
replicate weights):
  - Nodes padded to 100352 = 8 x 12544; core d owns dst rows [d*SH, (d+1)*SH).
  - Aggregation commutes with the dense projections: each layer gathers
    projected rows h[src] (dma_gather, 8192 idxs/instruction), scales them by
    the folded GCN norm coefficient (DVE), and scatter-adds per dst block with
    a one-hot matmul on the TensorEngine accumulating in PSUM.
  - Layer tables are stored p-major ([128, 98*64]: node t*128+p at partition p,
    cols t*64..): projections write p-major directly (bias folded into the
    matmul via a ones row), the self-loop stream is read straight from SBUF,
    and the AllGather is split into 4 partition-quarters so the next pass's
    gathers (grouped by src quarter, s-outer sweeps) overlap the collective.
  - mu/logstd share one aggregation pass (both use the unweighted norm on h2).
"""

import math

import numpy as np

import concourse.bass as bass
import concourse.bacc as bacc
import concourse.mybir as mybir
import concourse.tile as tile
from concourse.bass_utils import run_bass_kernel_spmd
from concourse.library_config import mlp

# ---- problem constants (hardcoded per contract) ----
N = 100000
FIN, HID, OUT = 128, 64, 32
NCORES = 8

# ---- layout constants ----
SH = 12544            # rows per core (100352 / 8)
NPAD = SH * NCORES    # 100352
NBLK = SH // 128      # 98 dst blocks per core
NSUB = 4              # src quarters (q = partition // 32); int16 gather idx
QROWS = 32            # partition rows per quarter
SUBR = NPAD // NSUB   # 25088 rows per quarter table
SLOTS = 1024          # gather slots per instruction (single_packet max: 64/engine)
CPG = SLOTS // 128    # chunks per gather group = 8
HCPG = CPG            # one-hot built in one op per group


def _wrap_idx(slots_i16):
    """[G*SLOTS] int16 -> [G, 128, SLOTS//16]: wrapped in 16 partitions, x8."""
    g = slots_i16.reshape(-1, SLOTS // 16, 16)
    g = np.swapaxes(g, 1, 2)
    return np.tile(g, (1, 8, 1)).astype(np.int16)


def _prep(edge_index, edge_weight):
    """Host-side edge partitioning. Returns (structure, per-core arrays)."""
    src = np.asarray(edge_index[0], dtype=np.int64)
    dst = np.asarray(edge_index[1], dtype=np.int64)
    ew = np.asarray(edge_weight, dtype=np.float32)

    deg_w = np.zeros(N, np.float32)
    np.add.at(deg_w, dst, ew)
    deg_w += 1.0  # self-loop weight
    deg_1 = (np.bincount(dst, minlength=N) + 1).astype(np.float32)
    dinv_w = 1.0 / np.sqrt(deg_w)
    dinv_1 = 1.0 / np.sqrt(deg_1)

    nw = dinv_w[src] * ew * dinv_w[dst]
    n1 = dinv_1[src] * dinv_1[dst]

    # merge duplicate (src, dst) pairs (self-loops handled via dense stream)
    key = src * NPAD + dst
    ukey, inv = np.unique(key, return_inverse=True)
    unw = np.zeros(len(ukey), np.float32)
    un1 = np.zeros(len(ukey), np.float32)
    np.add.at(unw, inv, nw)
    np.add.at(un1, inv, n1)
    usrc = ukey // NPAD
    udst = ukey % NPAD

    # src side: p-major quarter-table position
    sd = usrc // SH
    st_ = (usrc % SH) // 128
    sp_ = (usrc % SH) % 128
    s_all = sp_ // QROWS
    sidx_all = sd * (QROWS * NBLK) + (sp_ % QROWS) * NBLK + st_  # < 25088

    # dst side
    core = udst // SH
    t_all = (udst % SH) // 128
    dloc_all = (udst % SH) % 128

    # per (core, t, s) edge counts -> shared chunk structure K_ts
    cell = (core * NBLK + t_all) * NSUB + s_all
    cnt = np.bincount(cell, minlength=NCORES * NBLK * NSUB).reshape(NCORES, NBLK, NSUB)
    K_ts = np.maximum(1, np.ceil(cnt.max(axis=0) / 128).astype(np.int64))  # [NBLK, NSUB]
    C_s = K_ts.sum(axis=0)                      # chunks per s-stream
    G_s = [int(math.ceil(int(c) / CPG)) for c in C_s]
    base_pos = np.zeros((NBLK, NSUB), np.int64)  # chunk stream position of (t,s)
    for s in range(NSUB):
        base_pos[:, s] = np.cumsum(np.concatenate([[0], K_ts[:-1, s]]))

    per_core = []
    order = np.lexsort((sidx_all, t_all, s_all, core))
    osidx, os_, ot, odloc, onw, on1, ocore = (
        sidx_all[order], s_all[order], t_all[order],
        dloc_all[order], unw[order], un1[order], core[order],
    )
    cstart = np.searchsorted(ocore, np.arange(NCORES + 1))
    for d in range(NCORES):
        lo, hi = cstart[d], cstart[d + 1]
        dt, ds = ot[lo:hi], os_[lo:hi]
        dsl, ddl = osidx[lo:hi], odloc[lo:hi]
        dnw, dn1 = onw[lo:hi], on1[lo:hi]
        idxw, dlocv, normv = [], [], []
        cell_d = ds * NBLK + dt
        cello = np.searchsorted(cell_d, np.arange(NSUB * NBLK + 1))
        for s in range(NSUB):
            nslot = G_s[s] * SLOTS
            sl = np.zeros(nslot, np.int64)
            dl = np.zeros(nslot, np.float32)
            wv = np.zeros(nslot, np.float32)
            v1 = np.zeros(nslot, np.float32)
            for t in range(NBLK):
                a, b = cello[s * NBLK + t], cello[s * NBLK + t + 1]
                n = b - a
                p0 = base_pos[t, s] * 128
                sl[p0:p0 + n] = dsl[a:b]
                dl[p0:p0 + n] = ddl[a:b]
                wv[p0:p0 + n] = dnw[a:b]
                v1[p0:p0 + n] = dn1[a:b]
            sl[int(C_s[s]) * 128:] = -1  # trailing: ucode skips descriptors
            G = G_s[s]
            idxw.append(_wrap_idx(sl.astype(np.int16)))
            dlocv.append(
                dl.reshape(G, CPG, 128).transpose(0, 2, 1).astype(np.float16).copy()
            )
            nv = np.empty((G, 128, 2 * CPG), np.float32)
            nv[:, :, :CPG] = wv.reshape(G, CPG, 128).transpose(0, 2, 1)
            nv[:, :, CPG:] = v1.reshape(G, CPG, 128).transpose(0, 2, 1)
            normv.append(nv)
        # self-loop norms, p-major: [p, t] = dinv^2 of row d*SH + t*128 + p
        selfn = np.zeros((128, 2 * NBLK), np.float32)
        v_glob = (d * SH + np.arange(SH, dtype=np.int64)).reshape(NBLK, 128)
        real = v_glob < N
        sw = np.zeros((NBLK, 128), np.float32)
        s1 = np.zeros((NBLK, 128), np.float32)
        sw[real] = (dinv_w * dinv_w)[v_glob[real]]
        s1[real] = (dinv_1 * dinv_1)[v_glob[real]]
        selfn[:, :NBLK] = sw.T
        selfn[:, NBLK:] = s1.T
        per_core.append((idxw, dlocv, normv, selfn))

    used_s = [
        [min(SLOTS, int(C_s[s]) * 128 - g * SLOTS) for g in range(G_s[s])]
        for s in range(NSUB)
    ]
    return K_ts, G_s, base_pos, used_s, per_core


def _build(K_ts, G_s, base_pos, used_s):
    f32 = mybir.dt.float32
    f16 = mybir.dt.float16
    i16 = mybir.dt.int16
    nc = bacc.Bacc(None, target_bir_lowering=False, num_swdge_queues=4, num_devices=NCORES)

    xs_d = nc.dram_tensor("xs", [SH, FIN], f32, kind="ExternalInput")
    idx_d = [nc.dram_tensor(f"idx{s}", [G_s[s], 128, SLOTS // 16], i16, kind="ExternalInput") for s in range(NSUB)]
    dloc_d = [nc.dram_tensor(f"dloc{s}", [G_s[s], 128, CPG], f16, kind="ExternalInput") for s in range(NSUB)]
    norm_d = [nc.dram_tensor(f"norm{s}", [G_s[s], 128, 2 * CPG], f32, kind="ExternalInput") for s in range(NSUB)]
    selfn_d = nc.dram_tensor("selfn", [128, 2 * NBLK], f32, kind="ExternalInput")
    w1_d = nc.dram_tensor("w1h", [FIN, HID], f16, kind="ExternalInput")
    rl_d = nc.dram_tensor("rlr", [HID + 1, HID], f16, kind="ExternalInput")
    w2r_d = nc.dram_tensor("w2r", [HID + 1, HID], f16, kind="ExternalInput")
    w3r_d = nc.dram_tensor("w3r", [HID + 1, 2 * OUT], f16, kind="ExternalInput")
    iota_d = nc.dram_tensor("iota", [128, HCPG * 128], f16, kind="ExternalInput")
    id32_d = nc.dram_tensor("id32", [128, 128], f32, kind="ExternalInput")
    id16_d = nc.dram_tensor("id16", [128, 128], f16, kind="ExternalInput")
    ones_d = nc.dram_tensor("ones", [1, NBLK * 128], f16, kind="ExternalInput")
    out_d = nc.dram_tensor("out", [128, NBLK * 2 * OUT], f16, kind="ExternalOutput")

    ag_in = [[nc.dram_tensor(f"ag{i}_{q}", [QROWS, NBLK * HID], f32) for q in range(NSUB)] for i in range(3)]
    tables = [[nc.dram_tensor(f"tab{i}_{q}", [SUBR, HID], f32, addr_space="Shared") for q in range(NSUB)] for i in range(3)]

    with tile.TileContext(nc) as tc:
        with (
            tc.tile_pool(name="const", bufs=1) as kpool,
            tc.tile_pool(name="idx", bufs=8) as ipool,
            tc.tile_pool(name="meta", bufs=8) as mpool,
            tc.tile_pool(name="g", bufs=8) as gpool,
            tc.tile_pool(name="gh", bufs=4) as hpool,
            tc.tile_pool(name="b", bufs=4) as bpool,
            tc.tile_pool(name="st", bufs=2) as spool,
            tc.tile_pool(name="selfd", bufs=1) as dpool,
            tc.tile_pool(name="o", bufs=1) as opool,
            tc.tile_pool(name="x", bufs=2) as xpool,
            tc.tile_pool(name="pagg", bufs=2, space="PSUM") as pagg,
            tc.tile_pool(name="pmm", bufs=2, space="PSUM") as pmm,
            tc.tile_pool(name="ptr", bufs=2, space="PSUM") as ptr,
        ):
            nc.gpsimd.load_library(mlp)

            iota_t = kpool.tile([128, HCPG * 128], f16)
            nc.sync.dma_start(iota_t[:], iota_d[:])
            id32_t = kpool.tile([128, 128], f32)
            nc.sync.dma_start(id32_t[:], id32_d[:])
            id16_t = kpool.tile([128, 128], f16)
            nc.sync.dma_start(id16_t[:], id16_d[:])
            w1_t = kpool.tile([FIN, HID], f16)
            nc.sync.dma_start(w1_t[:], w1_d[:])
            rl_t = kpool.tile([HID + 1, HID], f16)
            nc.sync.dma_start(rl_t[:], rl_d[:])
            w2r_t = kpool.tile([HID + 1, HID], f16)
            nc.sync.dma_start(w2r_t[:], w2r_d[:])
            w3r_t = kpool.tile([HID + 1, 2 * OUT], f16)
            nc.sync.dma_start(w3r_t[:], w3r_d[:])
            selfn_t = kpool.tile([128, 2 * NBLK], f32)
            nc.sync.dma_start(selfn_t[:], selfn_d[:])
            # aggregation accumulator with a constant ones row (bias input)
            agg_t = kpool.tile([HID + 1, NBLK * 128], f16)
            nc.sync.dma_start(agg_t[HID:HID + 1, :], ones_d[:])

            gq = [0]

            def ensure(cur, i, s, g, use_n1):
                if s in cur and cur[s][0] == g:
                    return cur[s][1]
                it = ipool.tile([128, SLOTS // 16], i16, tag="idx")
                nc.sync.dma_start(it[:], idx_d[s][g])
                dt_ = mpool.tile([128, CPG], f16, tag="dl")
                nc.sync.dma_start(dt_[:], dloc_d[s][g])
                nt = mpool.tile([128, 2 * CPG], f32, tag="nm")
                nc.sync.dma_start(nt[:], norm_d[s][g])
                gt = gpool.tile([128, CPG, HID], f32, tag="gt")
                nc.gpsimd.dma_gather(
                    gt[:], tables[i][s][:], it[:],
                    SLOTS, int(used_s[s][g]), HID, queue_num=gq[0] % 4,
                )
                gq[0] += 1
                uc = (int(used_s[s][g]) + 127) // 128  # chunks actually gathered
                nsl = nt[:, CPG:CPG + uc] if use_n1 else nt[:, :uc]
                gh = hpool.tile([128, CPG, HID], f16, tag="gh")
                nc.vector.tensor_tensor(
                    out=gh[:, :uc, :], in0=gt[:, :uc, :],
                    in1=nsl.to_broadcast([128, uc, HID]),
                    op=mybir.AluOpType.mult,
                )
                bt = bpool.tile([128, CPG, 128], f16, tag="bt")
                for h in range(CPG // HCPG):
                    nc.vector.tensor_tensor(
                        out=bt[:, h * HCPG:(h + 1) * HCPG, :],
                        in0=iota_t[:].rearrange("p (j v) -> p j v", j=HCPG),
                        in1=dt_[:, h * HCPG:(h + 1) * HCPG].to_broadcast([128, HCPG, 128]),
                        op=mybir.AluOpType.is_equal,
                    )
                cur[s] = (g, (gh, bt))
                return gh, bt

            def aggregate(i, st_prev, use_n1):
                """agg_t[:HID, t*128+p] = sum of coeff*table_i[src] into (t,p)."""
                dsh = dpool.tile([128, NBLK, HID], f16, tag="dsh")
                sn = selfn_t[:, NBLK:] if use_n1 else selfn_t[:, :NBLK]
                nc.vector.tensor_tensor(
                    out=dsh[:],
                    in0=st_prev[:].rearrange("p (t f) -> p t f", f=HID),
                    in1=sn.to_broadcast([128, NBLK, HID]),
                    op=mybir.AluOpType.mult,
                )
                cur = {}
                for s in range(NSUB):
                    for t in range(NBLK):
                        ps = pagg.tile([HID, 128], f32, tag="ps")
                        nchunks = int(K_ts[t, s])
                        if s == 0:
                            nc.tensor.matmul(
                                ps[:], lhsT=dsh[:, t, :], rhs=id16_t[:],
                                start=True, stop=False,
                            )
                        for k in range(nchunks):
                            pos = int(base_pos[t, s]) + k
                            g, j = divmod(pos, CPG)
                            gh, bt = ensure(cur, i, s, g, use_n1)
                            nc.tensor.matmul(
                                ps[:], lhsT=gh[:, j, :], rhs=bt[:, j, :],
                                start=(s != 0 and k == 0),
                                stop=(k == nchunks - 1),
                            )
                        dst = agg_t[:HID, t * 128:(t + 1) * 128]
                        if s == 0:
                            nc.scalar.activation(dst, ps[:], mybir.ActivationFunctionType.Copy)
                        else:
                            nc.vector.tensor_tensor(out=dst, in0=dst, in1=ps[:], op=mybir.AluOpType.add)

            def project(rhs_t, func, st_out, outw):
                """st_out[p, t*outw+f] = func((agg | ones).T @ rhs)[p, f]."""
                for t in range(NBLK):
                    pm = pmm.tile([128, outw], f32, tag="pm")
                    nc.tensor.matmul(
                        pm[:], lhsT=agg_t[:, t * 128:(t + 1) * 128], rhs=rhs_t[:],
                        start=True, stop=True,
                    )
                    nc.scalar.activation(st_out[:, t * outw:(t + 1) * outw], pm[:], func)

            def stores_and_ag(i, st_t):
                for q in range(NSUB):
                    nc.sync.dma_start(ag_in[i][q][:], st_t[q * QROWS:(q + 1) * QROWS, :])
                    nc.gpsimd.collective_compute(
                        "AllGather", mybir.AluOpType.bypass,
                        replica_groups=[list(range(NCORES))],
                        ins=[ag_in[i][q][:]], outs=[tables[i][q][:]],
                    )

            # ---- pre-projection: st0 = (x @ W1) p-major ----
            st0 = spool.tile([128, NBLK * HID], f32, tag="st")
            for t in range(NBLK):
                xt = xpool.tile([128, FIN], f32, tag="xt")
                nc.sync.dma_start(xt[:], xs_d[t * 128:(t + 1) * 128, :])
                ptx = ptr.tile([128, 128], f32, tag="ptx")
                nc.tensor.transpose(ptx[:], xt[:], id32_t[:])
                xT = xpool.tile([128, 128], f16, tag="xT")
                nc.vector.tensor_copy(xT[:], ptx[:])
                ph = pmm.tile([128, HID], f32, tag="pm")
                nc.tensor.matmul(ph[:], lhsT=xT[:], rhs=w1_t[:], start=True, stop=True)
                nc.scalar.activation(st0[:, t * HID:(t + 1) * HID], ph[:], mybir.ActivationFunctionType.Copy)
            stores_and_ag(0, st0)

            # ---- layer 1: aggregate projected x, then bias+relu ----
            aggregate(0, st0, use_n1=False)
            st1 = spool.tile([128, NBLK * HID], f32, tag="st")
            project(rl_t, mybir.ActivationFunctionType.Relu, st1, HID)
            stores_and_ag(1, st1)

            # ---- layer 2: aggregate h1, then W2 + bias ----
            aggregate(1, st1, use_n1=False)
            st2 = spool.tile([128, NBLK * HID], f32, tag="st")
            project(w2r_t, None or mybir.ActivationFunctionType.Copy, st2, HID)
            stores_and_ag(2, st2)

            # ---- layer 3: aggregate h2; mu/ls joint projection ----
            aggregate(2, st2, use_n1=True)
            st3 = opool.tile([128, NBLK * 2 * OUT], f16, tag="st3")
            project(w3r_t, mybir.ActivationFunctionType.Copy, st3, 2 * OUT)
            nc.sync.dma_start(out_d[:], st3[:])

    # Tile round-robins Pool-DMA completion sems over 8 DMASW lanes without
    # queue awareness, but each sem is hardware-locked to the first SWDGE
    # queue that increments it. Rewrite each gather's queue to lane % 4 so
    # every lane's sem is only ever incremented from one queue.
    for fn in nc.m.functions:
        for blk in fn.blocks:
            for ins in blk.instructions:
                if isinstance(ins, mybir.InstDMAGatherAnt) and ins.sync_info:
                    for u in ins.sync_info.on_update:
                        name = getattr(u, "ant_name", "") or ""
                        if name.startswith("DMASW"):
                            ins.queue_num = int(name[5:].split("_")[0]) % 4
                            break

    nc.compile()
    return nc


def _run(inputs, trace=False):
    x = np.asarray(inputs["x"], np.float32)
    K_ts, G_s, base_pos, used_s, per_core = _prep(
        np.asarray(inputs["edge_index"]), np.asarray(inputs["edge_weight"])
    )
    nc = _build(K_ts, G_s, base_pos, used_s)

    x_pad = np.zeros((NPAD, FIN), np.float32)
    x_pad[:N] = x

    W1 = np.asarray(inputs["W1"], np.float32)
    W2 = np.asarray(inputs["W2"], np.float32)
    Wmu = np.asarray(inputs["Wmu"], np.float32)
    Wls = np.asarray(inputs["Wls"], np.float32)
    b1 = np.asarray(inputs["b1"], np.float32)
    b2 = np.asarray(inputs["b2"], np.float32)
    bmu = np.asarray(inputs["bmu"], np.float32)
    bls = np.asarray(inputs["bls"], np.float32)

    rl = np.vstack([np.eye(HID, dtype=np.float32), b1[None, :]]).astype(np.float16)
    w2r = np.vstack([W2, b2[None, :]]).astype(np.float16)
    w3r = np.vstack(
        [np.hstack([Wmu, Wls]), np.hstack([bmu, bls])[None, :]]
    ).astype(np.float16)
    iota = np.tile(np.arange(128, dtype=np.float16)[None, :], (128, HCPG))

    shared = {
        "w1h": W1.astype(np.float16),
        "rlr": rl,
        "w2r": w2r,
        "w3r": w3r,
        "iota": iota.reshape(128, HCPG * 128),
        "id32": np.eye(128, dtype=np.float32),
        "id16": np.eye(128, dtype=np.float16),
        "ones": np.ones((1, NBLK * 128), np.float16),
    }
    in_maps = []
    for d in range(NCORES):
        idxw, dlocv, normv, selfn = per_core[d]
        m = dict(shared)
        m["xs"] = x_pad[d * SH:(d + 1) * SH]
        m["selfn"] = selfn
        for s in range(NSUB):
            m[f"idx{s}"] = idxw[s]
            m[f"dloc{s}"] = dlocv[s]
            m[f"norm{s}"] = normv[s]
        in_maps.append(m)

    res = run_bass_kernel_spmd(nc, in_maps, core_ids=list(range(NCORES)), trace=trace)
    # out: [128, NBLK*2*OUT] p-major -> rows t*128+p
    full = np.concatenate(
        [
            res.results[d]["out"]
            .reshape(128, NBLK, 2 * OUT)
            .transpose(1, 0, 2)
            .reshape(SH, 2 * OUT)
            for d in range(NCORES)
        ],
        axis=0,
    ).astype(np.float32)
    mu = full[:N, :OUT].copy()
    logstd = full[:N, OUT:].copy()
    return (mu, logstd), res


def kernel(**inputs):
    (mu, logstd), _ = _run(inputs, trace=False)
    return mu, logstd


# revision 14
# speedup vs baseline: 2.8015x; 2.3972x over previous
"""GCN VGAE encoder (nn_Encoder_25065429139538) on 8 Trainium2 NeuronCores.

Strategy (sharding_hint: shard nodes across cores, partition edges by dst,
replicate weights):
  - Nodes padded to 100352 = 8 x 12544; core d owns dst rows [d*SH, (d+1)*SH).
  - Aggregation commutes with the dense projections: each layer gathers
    projected rows h[src] (dma_gather, 1024 idxs/instruction = the
    single-packet ceiling, 4 SWDGE queues), scales them by the folded GCN norm
    (DVE), and scatter-adds per dst block with a one-hot matmul on the
    TensorEngine accumulating in PSUM (t-major; all 4 src-subtable streams
    live so each dst block flushes PSUM once, via ScalarE).
  - Layer tables are p-major ([128, 98*64]: node t*128+p at partition p, cols
    t*64..): projections write p-major directly via K=65 matmuls (bias folded
    in as a ones row of the accumulator), the self-loop stream is read
    straight from SBUF, and per-group metadata (idx + dloc + norms) is one
    packed DMA with bitcast views.
  - mu/logstd share one aggregation pass (both use the unweighted norm on h2).
"""

import math

import numpy as np

import concourse.bass as bass
import concourse.bacc as bacc
import concourse.mybir as mybir
import concourse.tile as tile
from concourse.bass_utils import run_bass_kernel_spmd
from concourse.library_config import mlp

# ---- problem constants (hardcoded per contract) ----
N = 100000
FIN, HID, OUT = 128, 64, 32
NCORES = 8

# ---- layout constants ----
SH = 12544            # rows per core (100352 / 8)
NPAD = SH * NCORES    # 100352
NBLK = SH // 128      # 98 dst blocks per core
NSUB = 4              # src subtables (core pairs); int16 gather idx < 25088
SUBR = NPAD // NSUB   # 25088 rows per subtable
SLOTS = 1024          # gather slots per instruction (single_packet max)
CPG = SLOTS // 128    # chunks per gather group = 8
MW = 104              # packed meta row: 64 idx i16 | 8 dloc f16 | 16+16 f32


def _wrap_idx(slots_i16):
    """[G*SLOTS] int16 -> [G, 128, SLOTS//16]: wrapped in 16 partitions, x8."""
    g = slots_i16.reshape(-1, SLOTS // 16, 16)
    g = np.swapaxes(g, 1, 2)
    return np.tile(g, (1, 8, 1)).astype(np.int16)


def _prep(edge_index, edge_weight):
    """Host-side edge partitioning. Returns (structure, per-core arrays)."""
    src = np.asarray(edge_index[0], dtype=np.int64)
    dst = np.asarray(edge_index[1], dtype=np.int64)
    ew = np.asarray(edge_weight, dtype=np.float32)

    deg_w = np.zeros(N, np.float32)
    np.add.at(deg_w, dst, ew)
    deg_w += 1.0  # self-loop weight
    deg_1 = (np.bincount(dst, minlength=N) + 1).astype(np.float32)
    dinv_w = 1.0 / np.sqrt(deg_w)
    dinv_1 = 1.0 / np.sqrt(deg_1)

    nw = dinv_w[src] * ew * dinv_w[dst]
    n1 = dinv_1[src] * dinv_1[dst]

    # merge duplicate (src, dst) pairs (self-loops handled via dense stream)
    key = src * NPAD + dst
    ukey, inv = np.unique(key, return_inverse=True)
    unw = np.zeros(len(ukey), np.float32)
    un1 = np.zeros(len(ukey), np.float32)
    np.add.at(unw, inv, nw)
    np.add.at(un1, inv, n1)
    usrc = ukey // NPAD
    udst = ukey % NPAD

    # src side: p-major position in the d-major AllGather table
    sd = usrc // SH
    st_ = (usrc % SH) // 128
    sp_ = (usrc % SH) % 128
    s_all = sd // 2
    sidx_all = (sd % 2) * SH + sp_ * NBLK + st_  # < 25088

    # dst side
    core = udst // SH
    t_all = (udst % SH) // 128
    dloc_all = (udst % SH) % 128

    # per (core, t, s) edge counts -> shared chunk structure K_ts
    cell = (core * NBLK + t_all) * NSUB + s_all
    cnt = np.bincount(cell, minlength=NCORES * NBLK * NSUB).reshape(NCORES, NBLK, NSUB)
    K_ts = np.maximum(1, np.ceil(cnt.max(axis=0) / 128).astype(np.int64))  # [NBLK, NSUB]
    C_s = K_ts.sum(axis=0)                      # chunks per s-stream
    G_s = [int(math.ceil(int(c) / CPG)) for c in C_s]
    base_pos = np.zeros((NBLK, NSUB), np.int64)  # chunk stream position of (t,s)
    for s in range(NSUB):
        base_pos[:, s] = np.cumsum(np.concatenate([[0], K_ts[:-1, s]]))

    per_core = []
    order = np.lexsort((sidx_all, t_all, s_all, core))
    osidx, os_, ot, odloc, onw, on1, ocore = (
        sidx_all[order], s_all[order], t_all[order],
        dloc_all[order], unw[order], un1[order], core[order],
    )
    cstart = np.searchsorted(ocore, np.arange(NCORES + 1))
    for d in range(NCORES):
        lo, hi = cstart[d], cstart[d + 1]
        dt, ds = ot[lo:hi], os_[lo:hi]
        dsl, ddl = osidx[lo:hi], odloc[lo:hi]
        dnw, dn1 = onw[lo:hi], on1[lo:hi]
        metav = []
        cell_d = ds * NBLK + dt
        cello = np.searchsorted(cell_d, np.arange(NSUB * NBLK + 1))
        for s in range(NSUB):
            nslot = G_s[s] * SLOTS
            sl = np.zeros(nslot, np.int64)
            dl = np.zeros(nslot, np.float32)
            wv = np.zeros(nslot, np.float32)
            v1 = np.zeros(nslot, np.float32)
            for t in range(NBLK):
                a, b = cello[s * NBLK + t], cello[s * NBLK + t + 1]
                n = b - a
                p0 = base_pos[t, s] * 128
                sl[p0:p0 + n] = dsl[a:b]
                dl[p0:p0 + n] = ddl[a:b]
                wv[p0:p0 + n] = dnw[a:b]
                v1[p0:p0 + n] = dn1[a:b]
            sl[int(C_s[s]) * 128:] = -1  # trailing: ucode skips descriptors
            G = G_s[s]
            mt = np.zeros((G, 128, MW), np.int16)
            mt[:, :, :64] = _wrap_idx(sl.astype(np.int16))
            mt[:, :, 64:72] = (
                dl.reshape(G, CPG, 128).transpose(0, 2, 1)
                .astype(np.float16).view(np.int16)
            )
            mt[:, :, 72:88] = (
                np.ascontiguousarray(wv.reshape(G, CPG, 128).transpose(0, 2, 1))
                .view(np.int16).reshape(G, 128, 16)
            )
            mt[:, :, 88:104] = (
                np.ascontiguousarray(v1.reshape(G, CPG, 128).transpose(0, 2, 1))
                .view(np.int16).reshape(G, 128, 16)
            )
            metav.append(mt)
        # self-loop norms, p-major: [p, t] = dinv^2 of row d*SH + t*128 + p
        selfn = np.zeros((128, 2 * NBLK), np.float32)
        v_glob = (d * SH + np.arange(SH, dtype=np.int64)).reshape(NBLK, 128)
        real = v_glob < N
        sw = np.zeros((NBLK, 128), np.float32)
        s1 = np.zeros((NBLK, 128), np.float32)
        sw[real] = (dinv_w * dinv_w)[v_glob[real]]
        s1[real] = (dinv_1 * dinv_1)[v_glob[real]]
        selfn[:, :NBLK] = sw.T
        selfn[:, NBLK:] = s1.T
        per_core.append((metav, selfn))

    used_s = [
        [min(SLOTS, int(C_s[s]) * 128 - g * SLOTS) for g in range(G_s[s])]
        for s in range(NSUB)
    ]
    return K_ts, G_s, base_pos, used_s, per_core


def _build(K_ts, G_s, base_pos, used_s):
    f32 = mybir.dt.float32
    f16 = mybir.dt.float16
    i16 = mybir.dt.int16
    nc = bacc.Bacc(None, target_bir_lowering=False, num_swdge_queues=4, num_devices=NCORES)

    xs_d = nc.dram_tensor("xs", [SH, FIN], f32, kind="ExternalInput")
    meta_d = [nc.dram_tensor(f"meta{s}", [G_s[s], 128, MW], i16, kind="ExternalInput") for s in range(NSUB)]
    selfn_d = nc.dram_tensor("selfn", [128, 2 * NBLK], f32, kind="ExternalInput")
    w1_d = nc.dram_tensor("w1h", [FIN, HID], f16, kind="ExternalInput")
    rl_d = nc.dram_tensor("rlr", [HID + 1, HID], f16, kind="ExternalInput")
    w2r_d = nc.dram_tensor("w2r", [HID + 1, HID], f16, kind="ExternalInput")
    w3r_d = nc.dram_tensor("w3r", [HID + 1, 2 * OUT], f16, kind="ExternalInput")
    iota_d = nc.dram_tensor("iota", [128, CPG * 128], f16, kind="ExternalInput")
    id32_d = nc.dram_tensor("id32", [128, 128], f32, kind="ExternalInput")
    id16_d = nc.dram_tensor("id16", [128, 128], f16, kind="ExternalInput")
    ones_d = nc.dram_tensor("ones", [1, NBLK * 128], f16, kind="ExternalInput")
    out_d = nc.dram_tensor("out", [128, NBLK * 2 * OUT], f16, kind="ExternalOutput")

    ag_in = [nc.dram_tensor(f"ag{i}", [128, NBLK * HID], f32) for i in range(3)]
    tables = [nc.dram_tensor(f"tab{i}", [NPAD, HID], f32, addr_space="Shared") for i in range(3)]

    with tile.TileContext(nc) as tc:
        with (
            tc.tile_pool(name="const", bufs=1) as kpool,
            tc.tile_pool(name="meta", bufs=4) as mpool,
            tc.tile_pool(name="g", bufs=4) as gpool,
            tc.tile_pool(name="gh", bufs=3) as hpool,
            tc.tile_pool(name="b", bufs=3) as bpool,
            tc.tile_pool(name="st", bufs=2) as spool,
            tc.tile_pool(name="selfd", bufs=1) as dpool,
            tc.tile_pool(name="o", bufs=1) as opool,
            tc.tile_pool(name="x", bufs=2) as xpool,
            tc.tile_pool(name="pagg", bufs=2, space="PSUM") as pagg,
            tc.tile_pool(name="pmm", bufs=2, space="PSUM") as pmm,
            tc.tile_pool(name="ptr", bufs=2, space="PSUM") as ptr,
        ):
            nc.gpsimd.load_library(mlp)

            iota_t = kpool.tile([128, CPG * 128], f16)
            nc.sync.dma_start(iota_t[:], iota_d[:])
            id32_t = kpool.tile([128, 128], f32)
            nc.sync.dma_start(id32_t[:], id32_d[:])
            id16_t = kpool.tile([128, 128], f16)
            nc.sync.dma_start(id16_t[:], id16_d[:])
            w1_t = kpool.tile([FIN, HID], f16)
            nc.sync.dma_start(w1_t[:], w1_d[:])
            rl_t = kpool.tile([HID + 1, HID], f16)
            nc.sync.dma_start(rl_t[:], rl_d[:])
            w2r_t = kpool.tile([HID + 1, HID], f16)
            nc.sync.dma_start(w2r_t[:], w2r_d[:])
            w3r_t = kpool.tile([HID + 1, 2 * OUT], f16)
            nc.sync.dma_start(w3r_t[:], w3r_d[:])
            selfn_t = kpool.tile([128, 2 * NBLK], f32)
            nc.sync.dma_start(selfn_t[:], selfn_d[:])
            # aggregation accumulator with a constant ones row (bias input)
            agg_t = kpool.tile([HID + 1, NBLK * 128], f16)
            nc.sync.dma_start(agg_t[HID:HID + 1, :], ones_d[:])

            gq = [0]

            def ensure(cur, i, s, g, use_n1):
                if s in cur and cur[s][0] == g:
                    return cur[s][1]
                mt = mpool.tile([128, MW], i16, tag=f"m{s}")
                nc.sync.dma_start(mt[:], meta_d[s][g])
                gt = gpool.tile([128, CPG, HID], f32, tag=f"g{s}")
                nc.gpsimd.dma_gather(
                    gt[:], tables[i][s * SUBR:(s + 1) * SUBR, :], mt[:, :64],
                    SLOTS, int(used_s[s][g]), HID, queue_num=gq[0] % 4,
                )
                gq[0] += 1
                uc = (int(used_s[s][g]) + 127) // 128  # chunks actually gathered
                nsl = (
                    mt[:, 88:104].bitcast(f32)[:, :uc]
                    if use_n1
                    else mt[:, 72:88].bitcast(f32)[:, :uc]
                )
                gh = hpool.tile([128, CPG, HID], f16, tag=f"h{s}")
                nc.vector.tensor_tensor(
                    out=gh[:, :uc, :], in0=gt[:, :uc, :],
                    in1=nsl.to_broadcast([128, uc, HID]),
                    op=mybir.AluOpType.mult,
                )
                bt = bpool.tile([128, CPG, 128], f16, tag=f"b{s}")
                nc.vector.tensor_tensor(
                    out=bt[:],
                    in0=iota_t[:].rearrange("p (j v) -> p j v", j=CPG),
                    in1=mt[:, 64:72].bitcast(f16).to_broadcast([128, CPG, 128]),
                    op=mybir.AluOpType.is_equal,
                )
                cur[s] = (g, (gh, bt))
                return gh, bt

            def aggregate(i, st_prev, use_n1):
                """agg_t[:HID, t*128+p] = sum of coeff*table_i[src] into (t,p)."""
                dsh = dpool.tile([128, NBLK, HID], f16, tag="dsh")
                sn = selfn_t[:, NBLK:] if use_n1 else selfn_t[:, :NBLK]
                nc.vector.tensor_tensor(
                    out=dsh[:],
                    in0=st_prev[:].rearrange("p (t f) -> p t f", f=HID),
                    in1=sn.to_broadcast([128, NBLK, HID]),
                    op=mybir.AluOpType.mult,
                )
                cur = {}
                for t in range(NBLK):
                    ps = pagg.tile([HID, 128], f32, tag="ps")
                    nchunks = int(K_ts[t].sum()) + 1
                    nc.tensor.matmul(
                        ps[:], lhsT=dsh[:, t, :], rhs=id16_t[:],
                        start=True, stop=False,
                    )
                    ci = 1
                    for s in range(NSUB):
                        for k in range(int(K_ts[t, s])):
                            pos = int(base_pos[t, s]) + k
                            g, j = divmod(pos, CPG)
                            gh, bt = ensure(cur, i, s, g, use_n1)
                            nc.tensor.matmul(
                                ps[:], lhsT=gh[:, j, :], rhs=bt[:, j, :],
                                start=False, stop=(ci == nchunks - 1),
                            )
                            ci += 1
                    nc.scalar.activation(
                        agg_t[:HID, t * 128:(t + 1) * 128], ps[:],
                        mybir.ActivationFunctionType.Copy,
                    )

            def project(rhs_t, func, st_out, outw):
                """st_out[p, t*outw+f] = func((agg | ones).T @ rhs)[p, f]."""
                for t in range(NBLK):
                    pm = pmm.tile([128, outw], f32, tag="pm")
                    nc.tensor.matmul(
                        pm[:], lhsT=agg_t[:, t * 128:(t + 1) * 128], rhs=rhs_t[:],
                        start=True, stop=True,
                    )
                    nc.scalar.activation(st_out[:, t * outw:(t + 1) * outw], pm[:], func)

            def stores_and_ag(i, st_t):
                nc.sync.dma_start(ag_in[i][:], st_t[:])
                nc.gpsimd.collective_compute(
                    "AllGather", mybir.AluOpType.bypass,
                    replica_groups=[list(range(NCORES))],
                    ins=[ag_in[i][:]], outs=[tables[i][:]],
                )

            # ---- pre-projection: st0 = (x @ W1) p-major ----
            st0 = spool.tile([128, NBLK * HID], f32, tag="st")
            for t in range(NBLK):
                xt = xpool.tile([128, FIN], f32, tag="xt")
                nc.sync.dma_start(xt[:], xs_d[t * 128:(t + 1) * 128, :])
                ptx = ptr.tile([128, 128], f32, tag="ptx")
                nc.tensor.transpose(ptx[:], xt[:], id32_t[:])
                xT = xpool.tile([128, 128], f16, tag="xT")
                nc.vector.tensor_copy(xT[:], ptx[:])
                ph = pmm.tile([128, HID], f32, tag="pm")
                nc.tensor.matmul(ph[:], lhsT=xT[:], rhs=w1_t[:], start=True, stop=True)
                nc.scalar.activation(st0[:, t * HID:(t + 1) * HID], ph[:], mybir.ActivationFunctionType.Copy)
            stores_and_ag(0, st0)

            # ---- layer 1: aggregate projected x, then bias+relu ----
            aggregate(0, st0, use_n1=False)
            st1 = spool.tile([128, NBLK * HID], f32, tag="st")
            project(rl_t, mybir.ActivationFunctionType.Relu, st1, HID)
            stores_and_ag(1, st1)

            # ---- layer 2: aggregate h1, then W2 + bias ----
            aggregate(1, st1, use_n1=False)
            st2 = spool.tile([128, NBLK * HID], f32, tag="st")
            project(w2r_t, mybir.ActivationFunctionType.Copy, st2, HID)
            stores_and_ag(2, st2)

            # ---- layer 3: aggregate h2; mu/ls joint projection ----
            aggregate(2, st2, use_n1=True)
            st3 = opool.tile([128, NBLK * 2 * OUT], f16, tag="st3")
            project(w3r_t, mybir.ActivationFunctionType.Copy, st3, 2 * OUT)
            nc.sync.dma_start(out_d[:], st3[:])

    # Tile round-robins Pool-DMA completion sems over 8 DMASW lanes without
    # queue awareness, but each sem is hardware-locked to the first SWDGE
    # queue that increments it. Rewrite each gather's queue to lane % 4 so
    # every lane's sem is only ever incremented from one queue.
    for fn in nc.m.functions:
        for blk in fn.blocks:
            for ins in blk.instructions:
                if isinstance(ins, mybir.InstDMAGatherAnt) and ins.sync_info:
                    for u in ins.sync_info.on_update:
                        name = getattr(u, "ant_name", "") or ""
                        if name.startswith("DMASW"):
                            ins.queue_num = int(name[5:].split("_")[0]) % 4
                            break

    nc.compile()
    return nc


def _run(inputs, trace=False):
    x = np.asarray(inputs["x"], np.float32)
    K_ts, G_s, base_pos, used_s, per_core = _prep(
        np.asarray(inputs["edge_index"]), np.asarray(inputs["edge_weight"])
    )
    nc = _build(K_ts, G_s, base_pos, used_s)

    x_pad = np.zeros((NPAD, FIN), np.float32)
    x_pad[:N] = x

    W1 = np.asarray(inputs["W1"], np.float32)
    W2 = np.asarray(inputs["W2"], np.float32)
    Wmu = np.asarray(inputs["Wmu"], np.float32)
    Wls = np.asarray(inputs["Wls"], np.float32)
    b1 = np.asarray(inputs["b1"], np.float32)
    b2 = np.asarray(inputs["b2"], np.float32)
    bmu = np.asarray(inputs["bmu"], np.float32)
    bls = np.asarray(inputs["bls"], np.float32)

    rl = np.vstack([np.eye(HID, dtype=np.float32), b1[None, :]]).astype(np.float16)
    w2r = np.vstack([W2, b2[None, :]]).astype(np.float16)
    w3r = np.vstack(
        [np.hstack([Wmu, Wls]), np.hstack([bmu, bls])[None, :]]
    ).astype(np.float16)
    iota = np.tile(np.arange(128, dtype=np.float16)[None, :], (128, CPG))

    shared = {
        "w1h": W1.astype(np.float16),
        "rlr": rl,
        "w2r": w2r,
        "w3r": w3r,
        "iota": iota.reshape(128, CPG * 128),
        "id32": np.eye(128, dtype=np.float32),
        "id16": np.eye(128, dtype=np.float16),
        "ones": np.ones((1, NBLK * 128), np.float16),
    }
    in_maps = []
    for d in range(NCORES):
        metav, selfn = per_core[d]
        m = dict(shared)
        m["xs"] = x_pad[d * SH:(d + 1) * SH]
        m["selfn"] = selfn
        for s in range(NSUB):
            m[f"meta{s}"] = metav[s]
        in_maps.append(m)

    res = run_bass_kernel_spmd(nc, in_maps, core_ids=list(range(NCORES)), trace=trace)
    # out: [128, NBLK*2*OUT] p-major -> rows t*128+p
    full = np.concatenate(
        [
            res.results[d]["out"]
            .reshape(128, NBLK, 2 * OUT)
            .transpose(1, 0, 2)
            .reshape(SH, 2 * OUT)
            for d in range(NCORES)
        ],
        axis=0,
    ).astype(np.float32)
    mu = full[:N, :OUT].copy()
    logstd = full[:N, OUT:].copy()
    return (mu, logstd), res


def kernel(**inputs):
    (mu, logstd), _ = _run(inputs, trace=False)
    return mu, logstd
